# revision 20
# baseline (speedup 1.0000x reference)
"""CapsNet routing layer (nn_CapsLayer) on 8 Trainium2 NeuronCores.

reference:
    u_hat = einsum("ncoi,bci->bnco", W[0], x)         # B,N,C,O = 1024,2,512,64
    3 dynamic-routing iterations (softmax over n, weighted sum over c,
    squash, agreement update); returns v from iteration 3.

Strategy (data-parallel over batch, hardcoded shapes):
  - 8 cores x 128 samples, host pre-transposed inputs so all DMAs are
    wide contiguous rows.
  - GEMM runs as a bf16 hi/lo x3 decomposition (xh*Wh + xh*Wl + xl*Wh,
    fp32 PSUM accumulate) -> ~1e-5 relative accuracy at bf16 PE rate
    with fast-weight-load (needs 128 stationary columns = full batch).
  - u_hat for samples 0..63 lands in SBUF (fp32, 128KB/partition);
    samples 64..127 spill PSUM->DRAM and reload after chunk 0 routes.
  - Routing is all on-chip: softmax over N=2 collapses to
    sigmoid(b0-b1); iteration 1 uses exact coefficients 0.5;
    iteration 3 skips the dead y/b update. Work is split DVE/GPSIMD.
"""
import os
import sys
import types

sys.path.insert(0, "/opt/trn_rl_repo")

import numpy as np
import ml_dtypes
import concourse.bass as bass
import concourse.mybir as mybir
import concourse.tile as tile
from concourse.bass_utils import run_bass_kernel_spmd

BF16NP = ml_dtypes.bfloat16

B, NCAPS, C, ICH, OCH = 1024, 2, 512, 256, 64
ITERATIONS = 3
NCORES = 8
BPC = B // NCORES            # batch per core = 128
NCHUNK = 2                   # routing chunks per core
BCH = BPC // NCHUNK          # batch per chunk = 64
KH = 2                       # K halves (ICH = 2*128)
CG = 4                       # c's per GEMM/DMA group
CSEG = 32                    # c's per routing tree segment
NSEG = C // CSEG             # 16
GP_SEGS = 6                  # segments routed on GPSIMD (rest on DVE)

FP32 = mybir.dt.float32
BF16 = mybir.dt.bfloat16
ADD = mybir.AluOpType.add
MULT = mybir.AluOpType.mult
SUB = mybir.AluOpType.subtract
AF = mybir.ActivationFunctionType

LAST_EXEC_NS = None


def _install_profile_hook():
    """antenv.axon_hooks is absent in this image; recreate it so
    run_bass_kernel_spmd(trace=True)/BASS_TRACE can report exec_time_ns."""
    if "antenv.axon_hooks" in sys.modules:
        return
    mod = types.ModuleType("antenv.axon_hooks")
    mod._hook = None
    mod.set_axon_ntff_profile_hook = lambda h: setattr(mod, "_hook", h)
    mod.get_axon_ntff_profile_hook = lambda: mod._hook
    sys.modules["antenv.axon_hooks"] = mod
    try:
        from trn_agent_boot.trn_boot import _ntff_profile_via_ctypes

        hook = _ntff_profile_via_ctypes("/opt/axon/libaxon_pjrt.so")
        if hook is not None:
            mod._hook = hook
    except Exception:
        pass


def _split_sync_waits(nc, max_waits=1):
    """walrus setupSyncWait rejects instructions with more than one sem
    wait; hoist extras onto same-engine InstNoOp's placed just before."""
    for f in nc.m.functions:
        for bb in f.blocks:
            out = []
            changed = False
            for inst in bb.instructions:
                si = inst.sync_info
                waits = list(si.on_wait) if si is not None and si.on_wait else []
                if len(waits) > max_waits:
                    extra, keep = waits[:-max_waits], waits[-max_waits:]
                    for g, w in enumerate(extra):
                        out.append(
                            mybir.InstNoOp(
                                name=f"{inst.name}_wsplit{g}",
                                engine=inst.engine,
                                bass_nofuse=True,
                                sync_info=mybir.SyncInfo(on_wait=[w], on_update=[]),
                            )
                        )
                    inst.sync_info = mybir.SyncInfo(
                        on_wait=keep,
                        on_update=list(si.on_update) if si.on_update else [],
                    )
                    changed = True
                out.append(inst)
            if changed:
                bb.instructions = out


def _tree_halve(eng, cur, axis, tmp_tiles, final_out, stop_at=1):
    """Binary tree reduce along `axis` (1 or 2) of a (128, A, B) AP by
    repeated halving down to `stop_at`. When stop_at==1 the last level
    writes final_out; otherwise returns the AP of size stop_at."""
    size = cur.shape[axis]
    bi = 0
    while size > stop_at:
        half = size // 2
        if axis == 1:
            in0, in1 = cur[:, 0:half, :], cur[:, half : 2 * half, :]
        else:
            in0, in1 = cur[:, :, 0:half], cur[:, :, half : 2 * half]
        if half == stop_at and stop_at == 1:
            dst = final_out
        else:
            t = tmp_tiles[bi % len(tmp_tiles)]
            if axis == 1:
                dst = t[:, 0:half, :]
            else:
                dst = t[:, :, 0:half]
        eng.tensor_tensor(dst, in0, in1, op=ADD)
        cur = dst
        size = half
        bi += 1
    return cur


def build_kernel(split_waits=True, debug_dump=False):
    nc = bass.Bass("TRN2", target_bir_lowering=False, debug=False)
    xth = nc.dram_tensor("xth", [KH, 128, C, BPC], BF16, kind="ExternalInput").ap()
    xtl = nc.dram_tensor("xtl", [KH, 128, C, BPC], BF16, kind="ExternalInput").ap()
    wth = nc.dram_tensor("wth", [KH, 128, C, NCAPS * OCH], BF16, kind="ExternalInput").ap()
    wtl = nc.dram_tensor("wtl", [KH, 128, C, NCAPS * OCH], BF16, kind="ExternalInput").ap()
    out = nc.dram_tensor("out", [BPC, NCAPS, OCH], FP32, kind="ExternalOutput").ap()
    usp = nc.dram_tensor("usp", [BCH, C, NCAPS, OCH], FP32).ap()  # chunk-1 spill
    if debug_dump:
        dbg_u = nc.dram_tensor("dbg_u", [128, C, OCH], FP32, kind="ExternalOutput").ap()

    with tile.TileContext(nc) as tc:
        with (
            tc.tile_pool(name="xin", bufs=2) as xpool,
            tc.tile_pool(name="win", bufs=2) as wpool,
            tc.tile_pool(name="psum", bufs=4, space="PSUM") as pspool,
            tc.tile_pool(name="ubuf", bufs=1) as upool,
            tc.tile_pool(name="scr", bufs=1) as scrpool,
        ):
            # static small tensors, shared across chunks (chunks serialize)
            coeff = nc.alloc_sbuf_tensor("coeff", [128, C], FP32).ap()
            d = nc.alloc_sbuf_tensor("d", [BCH, C], FP32).ap()
            y0 = nc.alloc_sbuf_tensor("y0", [BCH, C], FP32).ap()
            y1 = nc.alloc_sbuf_tensor("y1", [BCH, C], FP32).ap()
            dy = nc.alloc_sbuf_tensor("dy", [BCH, C], FP32).ap()
            partial = nc.alloc_sbuf_tensor("partial", [128, NSEG, OCH], FP32).ap()
            pc8 = nc.alloc_sbuf_tensor("pc8", [128, 8, OCH], FP32).ap()
            s = nc.alloc_sbuf_tensor("s", [128, OCH], FP32).ap()
            s2 = nc.alloc_sbuf_tensor("s2", [128, OCH], FP32).ap()
            sq = nc.alloc_sbuf_tensor("sq", [128, 1], FP32).ap()
            aa = nc.alloc_sbuf_tensor("aa", [128, 1], FP32).ap()
            rr = nc.alloc_sbuf_tensor("rr", [128, 1], FP32).ap()
            den = nc.alloc_sbuf_tensor("den", [128, 1], FP32).ap()
            rec = nc.alloc_sbuf_tensor("rec", [128, 1], FP32).ap()
            mm = nc.alloc_sbuf_tensor("mm", [128, 1], FP32).ap()
            v = nc.alloc_sbuf_tensor("v", [128, OCH], FP32).ap()
            pa = nc.alloc_sbuf_tensor("pa", [128, 4, OCH], FP32).ap()
            pb = nc.alloc_sbuf_tensor("pb", [128, 2, OCH], FP32).ap()

            # ---- GEMM (all 128 b at once, bf16 hi/lo x3) ----
            u = upool.tile([128, C, OCH], FP32, tag="u")
            for cg in range(C // CG):
                c0 = cg * CG
                xh, xl, wh, wl = [], [], [], []
                for h in range(KH):
                    th = xpool.tile([128, CG, BPC], BF16, tag=f"xh{h}")
                    nc.sync.dma_start(th[:], xth[h, :, c0 : c0 + CG, :])
                    xh.append(th)
                    tl = xpool.tile([128, CG, BPC], BF16, tag=f"xl{h}")
                    nc.sync.dma_start(tl[:], xtl[h, :, c0 : c0 + CG, :])
                    xl.append(tl)
                    twh = wpool.tile([128, CG, NCAPS * OCH], BF16, tag=f"wh{h}")
                    nc.sync.dma_start(twh[:], wth[h, :, c0 : c0 + CG, :])
                    wh.append(twh)
                    twl = wpool.tile([128, CG, NCAPS * OCH], BF16, tag=f"wl{h}")
                    nc.sync.dma_start(twl[:], wtl[h, :, c0 : c0 + CG, :])
                    wl.append(twl)
                pg = pspool.tile([128, CG, NCAPS * OCH], FP32, tag="pg")
                for j in range(CG):
                    first = True
                    for h in range(KH):
                        for sta, mov in ((xh[h], wh[h]), (xh[h], wl[h]), (xl[h], wh[h])):
                            nc.tensor.matmul(
                                pg[:, j, :],
                                lhsT=sta[:, j, :],
                                rhs=mov[:, j, :],
                                start=first,
                                stop=(h == KH - 1 and sta is xl[h]),
                            )
                            first = False
                # chunk 0 rows -> u tile; chunk 1 rows -> DRAM spill (via SBUF)
                nc.scalar.copy(u[0:BCH, c0 : c0 + CG, :], pg[0:BCH, :, 0:OCH])
                nc.scalar.copy(
                    u[BCH:128, c0 : c0 + CG, :], pg[0:BCH, :, OCH : 2 * OCH]
                )
                stg = xpool.tile([BCH, CG, NCAPS * OCH], FP32, tag="stg")
                nc.scalar.copy(stg[:], pg[BCH:128, :, :])
                nc.sync.dma_start(usp[:, c0 : c0 + CG, :, :], stg[:])

            for chunk in range(NCHUNK):
                if chunk > 0:
                    u = upool.tile([128, C, OCH], FP32, tag="u")
                    nc.sync.dma_start(u[0:BCH, :, :], usp[:, :, 0, :])
                    nc.sync.dma_start(u[BCH:128, :, :], usp[:, :, 1, :])

                for it in range(ITERATIONS):
                    # --- s = sum_c coeff * u  (coeff == 0.5 on iteration 0) ---
                    for seg in range(NSEG):
                        gp = seg < GP_SEGS
                        eng = nc.gpsimd if gp else nc.vector
                        ek = "g" if gp else "v"
                        cs = seg * CSEG
                        useg = u[:, cs : cs + CSEG, :]
                        if it == 0:
                            src = useg
                            w = scrpool.tile([128, CSEG, OCH], FP32, tag=f"w{ek}")
                        else:
                            w = scrpool.tile([128, CSEG, OCH], FP32, tag=f"w{ek}")
                            cb = (
                                coeff[:, cs : cs + CSEG]
                                .unsqueeze(2)
                                .broadcast_to((128, CSEG, OCH))
                            )
                            eng.tensor_tensor(w[:], useg, cb, op=MULT)
                            src = w[:]
                        ta = scrpool.tile([128, CSEG // 2, OCH], FP32, tag=f"ta{ek}")
                        _tree_halve(eng, src, 1, [ta, w], partial[:, seg : seg + 1, :])
                    # partial (128, 16, O) -> s
                    nc.vector.tensor_tensor(
                        pc8[:], partial[:, 0:8, :], partial[:, 8:16, :], op=ADD
                    )
                    nc.vector.tensor_tensor(pa[:], pc8[:, 0:4, :], pc8[:, 4:8, :], op=ADD)
                    nc.vector.tensor_tensor(pb[:], pa[:, 0:2, :], pa[:, 2:4, :], op=ADD)
                    nc.vector.tensor_tensor(
                        s[:].unsqueeze(1), pb[:, 0:1, :], pb[:, 1:2, :], op=ADD
                    )
                    if it == 0:
                        nc.vector.tensor_scalar(s[:], s[:], 0.5, None, op0=MULT)

                    # --- squash: v = s * sq / ((1+sq) sqrt(sq)) ---
                    nc.vector.tensor_tensor(s2[:], s[:], s[:], op=MULT)
                    nc.vector.tensor_reduce(sq[:], s2[:], axis=mybir.AxisListType.X, op=ADD)
                    nc.vector.tensor_scalar(aa[:], sq[:], 1.0, None, op0=ADD)
                    nc.scalar.activation(rr[:], sq[:], AF.Sqrt)
                    nc.vector.tensor_tensor(den[:], aa[:], rr[:], op=MULT)
                    nc.vector.reciprocal(rec[:], den[:])
                    nc.vector.tensor_tensor(mm[:], sq[:], rec[:], op=MULT)
                    nc.vector.tensor_scalar(v[:], s[:], mm[:], None, op0=MULT)

                    if it == ITERATIONS - 1:
                        break

                    # --- y[(n,b),c] = sum_o u * v ---
                    vb = v[:].unsqueeze(1).broadcast_to((128, CSEG, OCH))
                    for seg in range(NSEG):
                        gp = seg < GP_SEGS
                        eng = nc.gpsimd if gp else nc.vector
                        ek = "g" if gp else "v"
                        cs = seg * CSEG
                        w = scrpool.tile([128, CSEG, OCH], FP32, tag=f"w{ek}")
                        eng.tensor_tensor(w[:], u[:, cs : cs + CSEG, :], vb, op=MULT)
                        ta = scrpool.tile([128, CSEG, OCH // 2], FP32, tag=f"ta{ek}")
                        cur = _tree_halve(eng, w[:], 2, [ta, w], None, stop_at=2)
                        eng.tensor_tensor(
                            y0[:, cs : cs + CSEG].unsqueeze(2),
                            cur[0:BCH, :, 0:1],
                            cur[0:BCH, :, 1:2],
                            op=ADD,
                        )
                        eng.tensor_tensor(
                            y1[:, cs : cs + CSEG].unsqueeze(2),
                            cur[BCH:128, :, 0:1],
                            cur[BCH:128, :, 1:2],
                            op=ADD,
                        )
                    # --- d += y0 - y1 ; coeff = sigmoid(+/- d) ---
                    if it == 0:
                        nc.vector.tensor_tensor(d[:], y0[:], y1[:], op=SUB)
                    else:
                        nc.vector.tensor_tensor(dy[:], y0[:], y1[:], op=SUB)
                        nc.vector.tensor_tensor(d[:], d[:], dy[:], op=ADD)
                    nc.scalar.activation(coeff[0:BCH, :], d[:], AF.Sigmoid)
                    nc.scalar.activation(coeff[BCH:128, :], d[:], AF.Sigmoid, scale=-1.0)

                # --- output: v (p=(n,b64), f=o) -> out[b, n, o] ---
                if debug_dump and chunk == 0:
                    nc.sync.dma_start(dbg_u[:], u[:])
                bs = chunk * BCH
                nc.sync.dma_start(out[bs : bs + BCH, 0, :], v[0:BCH, :])
                nc.sync.dma_start(out[bs : bs + BCH, 1, :], v[BCH:128, :])

    if split_waits:
        _split_sync_waits(nc)
    return nc


def _prep_inputs(x, W):
    x = np.ascontiguousarray(x, dtype=np.float32)
    W0 = np.ascontiguousarray(W.reshape(NCAPS, C, OCH, ICH), dtype=np.float32)
    # wt[h, i, c, n*64+o] = W0[n, c, o, h*128+i]
    w4 = W0.reshape(NCAPS, C, OCH, KH, 128)
    wt = np.ascontiguousarray(w4.transpose(3, 4, 1, 0, 2)).reshape(
        KH, 128, C, NCAPS * OCH
    )
    wth = wt.astype(BF16NP)
    wtl = (wt - wth.astype(np.float32)).astype(BF16NP)
    xth_cores, xtl_cores = [], []
    for k in range(NCORES):
        xc = x[k * BPC : (k + 1) * BPC]  # (128, C, ICH)
        x5 = xc.reshape(BPC, C, KH, 128)
        xt = np.ascontiguousarray(x5.transpose(2, 3, 1, 0))  # (h,i,c,b)
        xh = xt.astype(BF16NP)
        xlo = (xt - xh.astype(np.float32)).astype(BF16NP)
        xth_cores.append(xh)
        xtl_cores.append(xlo)
    return xth_cores, xtl_cores, wth, wtl


_NC_CACHE = {}


def kernel(x, W):
    global LAST_EXEC_NS
    _install_profile_hook()
    if "nc" not in _NC_CACHE:
        _NC_CACHE["nc"] = build_kernel()
    nc = _NC_CACHE["nc"]
    xth_cores, xtl_cores, wth, wtl = _prep_inputs(np.asarray(x), np.asarray(W))
    in_maps = [
        {"xth": xth_cores[k], "xtl": xtl_cores[k], "wth": wth, "wtl": wtl}
        for k in range(NCORES)
    ]
    trace = bool(os.environ.get("CAPS_TRACE"))
    res = run_bass_kernel_spmd(nc, in_maps, list(range(NCORES)), trace=trace)
    LAST_EXEC_NS = res.exec_time_ns
    out = np.concatenate([res.results[k]["out"] for k in range(NCORES)], axis=0)
    return out.astype(np.float32)


# revision 21
# speedup vs baseline: 1.0283x; 1.0283x over previous
"""CapsNet routing layer (nn_CapsLayer) on 8 Trainium2 NeuronCores.

reference:
    u_hat = einsum("ncoi,bci->bnco", W[0], x)         # B,N,C,O = 1024,2,512,64
    3 dynamic-routing iterations (softmax over n, weighted sum over c,
    squash, agreement update); returns v from iteration 3.

Strategy (data-parallel over batch, hardcoded shapes):
  - 8 cores x 128 samples, host pre-transposed inputs so all DMAs are
    wide contiguous rows.
  - GEMM runs as a bf16 hi/lo x3 decomposition (xh*Wh + xh*Wl + xl*Wh,
    fp32 PSUM accumulate) -> ~1e-5 relative accuracy at bf16 PE rate
    with fast-weight-load (needs 128 stationary columns = full batch).
  - u_hat for samples 0..63 lands in SBUF (fp32, 128KB/partition);
    samples 64..127 spill PSUM->DRAM and reload after chunk 0 routes.
  - Routing is all on-chip: softmax over N=2 collapses to
    sigmoid(b0-b1); iteration 1 uses exact coefficients 0.5;
    iteration 3 skips the dead y/b update. Work is split DVE/GPSIMD.
"""
import os
import sys
import types

sys.path.insert(0, "/opt/trn_rl_repo")

import numpy as np
import ml_dtypes
import concourse.bass as bass
import concourse.mybir as mybir
import concourse.tile as tile
from concourse.bass_utils import run_bass_kernel_spmd

BF16NP = ml_dtypes.bfloat16

B, NCAPS, C, ICH, OCH = 1024, 2, 512, 256, 64
ITERATIONS = 3
NCORES = 8
BPC = B // NCORES            # batch per core = 128
NCHUNK = 2                   # routing chunks per core
BCH = BPC // NCHUNK          # batch per chunk = 64
KH = 2                       # K halves (ICH = 2*128)
CG = 4                       # c's per GEMM/DMA group
CSEG = 32                    # c's per routing tree segment
NSEG = C // CSEG             # 16
GP_SEGS = 0                  # segments routed on GPSIMD (rest on DVE)

FP32 = mybir.dt.float32
BF16 = mybir.dt.bfloat16
ADD = mybir.AluOpType.add
MULT = mybir.AluOpType.mult
SUB = mybir.AluOpType.subtract
AF = mybir.ActivationFunctionType

LAST_EXEC_NS = None


def _install_profile_hook():
    """antenv.axon_hooks is absent in this image; recreate it so
    run_bass_kernel_spmd(trace=True)/BASS_TRACE can report exec_time_ns."""
    if "antenv.axon_hooks" in sys.modules:
        return
    mod = types.ModuleType("antenv.axon_hooks")
    mod._hook = None
    mod.set_axon_ntff_profile_hook = lambda h: setattr(mod, "_hook", h)
    mod.get_axon_ntff_profile_hook = lambda: mod._hook
    sys.modules["antenv.axon_hooks"] = mod
    try:
        from trn_agent_boot.trn_boot import _ntff_profile_via_ctypes

        hook = _ntff_profile_via_ctypes("/opt/axon/libaxon_pjrt.so")
        if hook is not None:
            mod._hook = hook
    except Exception:
        pass


def _split_sync_waits(nc, max_waits=1):
    """walrus setupSyncWait rejects instructions with more than one sem
    wait; hoist extras onto same-engine InstNoOp's placed just before."""
    for f in nc.m.functions:
        for bb in f.blocks:
            out = []
            changed = False
            for inst in bb.instructions:
                si = inst.sync_info
                waits = list(si.on_wait) if si is not None and si.on_wait else []
                if len(waits) > max_waits:
                    extra, keep = waits[:-max_waits], waits[-max_waits:]
                    for g, w in enumerate(extra):
                        out.append(
                            mybir.InstNoOp(
                                name=f"{inst.name}_wsplit{g}",
                                engine=inst.engine,
                                bass_nofuse=True,
                                sync_info=mybir.SyncInfo(on_wait=[w], on_update=[]),
                            )
                        )
                    inst.sync_info = mybir.SyncInfo(
                        on_wait=keep,
                        on_update=list(si.on_update) if si.on_update else [],
                    )
                    changed = True
                out.append(inst)
            if changed:
                bb.instructions = out


def _tree_halve(eng, cur, axis, tmp_tiles, final_out, stop_at=1):
    """Binary tree reduce along `axis` (1 or 2) of a (128, A, B) AP by
    repeated halving down to `stop_at`. When stop_at==1 the last level
    writes final_out; otherwise returns the AP of size stop_at."""
    size = cur.shape[axis]
    bi = 0
    while size > stop_at:
        half = size // 2
        if axis == 1:
            in0, in1 = cur[:, 0:half, :], cur[:, half : 2 * half, :]
        else:
            in0, in1 = cur[:, :, 0:half], cur[:, :, half : 2 * half]
        if half == stop_at and stop_at == 1:
            dst = final_out
        else:
            t = tmp_tiles[bi % len(tmp_tiles)]
            if axis == 1:
                dst = t[:, 0:half, :]
            else:
                dst = t[:, :, 0:half]
        eng.tensor_tensor(dst, in0, in1, op=ADD)
        cur = dst
        size = half
        bi += 1
    return cur


def build_kernel(split_waits=True, debug_dump=False):
    nc = bass.Bass("TRN2", target_bir_lowering=False, debug=False)
    xth = nc.dram_tensor("xth", [KH, 128, C, BPC], BF16, kind="ExternalInput").ap()
    xtl = nc.dram_tensor("xtl", [KH, 128, C, BPC], BF16, kind="ExternalInput").ap()
    wth = nc.dram_tensor("wth", [KH, 128, C, NCAPS * OCH], BF16, kind="ExternalInput").ap()
    wtl = nc.dram_tensor("wtl", [KH, 128, C, NCAPS * OCH], BF16, kind="ExternalInput").ap()
    out = nc.dram_tensor("out", [BPC, NCAPS, OCH], FP32, kind="ExternalOutput").ap()
    usp = nc.dram_tensor("usp", [BCH, C, NCAPS, OCH], FP32).ap()  # chunk-1 spill
    if debug_dump:
        dbg_u = nc.dram_tensor("dbg_u", [128, C, OCH], FP32, kind="ExternalOutput").ap()

    with tile.TileContext(nc) as tc:
        with (
            tc.tile_pool(name="xin", bufs=2) as xpool,
            tc.tile_pool(name="win", bufs=2) as wpool,
            tc.tile_pool(name="psum", bufs=4, space="PSUM") as pspool,
            tc.tile_pool(name="ubuf", bufs=1) as upool,
            tc.tile_pool(name="scr", bufs=1) as scrpool,
        ):
            # static small tensors, shared across chunks (chunks serialize)
            coeff = nc.alloc_sbuf_tensor("coeff", [128, C], FP32).ap()
            d = nc.alloc_sbuf_tensor("d", [BCH, C], FP32).ap()
            y0 = nc.alloc_sbuf_tensor("y0", [BCH, C], FP32).ap()
            y1 = nc.alloc_sbuf_tensor("y1", [BCH, C], FP32).ap()
            dy = nc.alloc_sbuf_tensor("dy", [BCH, C], FP32).ap()
            partial = nc.alloc_sbuf_tensor("partial", [128, NSEG, OCH], FP32).ap()
            pc8 = nc.alloc_sbuf_tensor("pc8", [128, 8, OCH], FP32).ap()
            s = nc.alloc_sbuf_tensor("s", [128, OCH], FP32).ap()
            s2 = nc.alloc_sbuf_tensor("s2", [128, OCH], FP32).ap()
            sq = nc.alloc_sbuf_tensor("sq", [128, 1], FP32).ap()
            aa = nc.alloc_sbuf_tensor("aa", [128, 1], FP32).ap()
            rr = nc.alloc_sbuf_tensor("rr", [128, 1], FP32).ap()
            den = nc.alloc_sbuf_tensor("den", [128, 1], FP32).ap()
            rec = nc.alloc_sbuf_tensor("rec", [128, 1], FP32).ap()
            mm = nc.alloc_sbuf_tensor("mm", [128, 1], FP32).ap()
            v = nc.alloc_sbuf_tensor("v", [128, OCH], FP32).ap()
            pa = nc.alloc_sbuf_tensor("pa", [128, 4, OCH], FP32).ap()
            pb = nc.alloc_sbuf_tensor("pb", [128, 2, OCH], FP32).ap()

            # ---- GEMM (all 128 b at once, bf16 hi/lo x3) ----
            u = upool.tile([128, C, OCH], FP32, tag="u")
            for cg in range(C // CG):
                c0 = cg * CG
                xh, xl, wh, wl = [], [], [], []
                for h in range(KH):
                    th = xpool.tile([128, CG, BPC], BF16, tag=f"xh{h}")
                    nc.sync.dma_start(th[:], xth[h, :, c0 : c0 + CG, :])
                    xh.append(th)
                    tl = xpool.tile([128, CG, BPC], BF16, tag=f"xl{h}")
                    nc.sync.dma_start(tl[:], xtl[h, :, c0 : c0 + CG, :])
                    xl.append(tl)
                    twh = wpool.tile([128, CG, NCAPS * OCH], BF16, tag=f"wh{h}")
                    nc.sync.dma_start(twh[:], wth[h, :, c0 : c0 + CG, :])
                    wh.append(twh)
                    twl = wpool.tile([128, CG, NCAPS * OCH], BF16, tag=f"wl{h}")
                    nc.sync.dma_start(twl[:], wtl[h, :, c0 : c0 + CG, :])
                    wl.append(twl)
                pg = pspool.tile([128, CG, NCAPS * OCH], FP32, tag="pg")
                for j in range(CG):
                    first = True
                    for h in range(KH):
                        for sta, mov in ((xh[h], wh[h]), (xh[h], wl[h]), (xl[h], wh[h])):
                            nc.tensor.matmul(
                                pg[:, j, :],
                                lhsT=sta[:, j, :],
                                rhs=mov[:, j, :],
                                start=first,
                                stop=(h == KH - 1 and sta is xl[h]),
                            )
                            first = False
                # chunk 0 rows -> u tile; chunk 1 rows -> DRAM spill (via SBUF)
                nc.scalar.copy(u[0:BCH, c0 : c0 + CG, :], pg[0:BCH, :, 0:OCH])
                nc.scalar.copy(
                    u[BCH:128, c0 : c0 + CG, :], pg[0:BCH, :, OCH : 2 * OCH]
                )
                stg = xpool.tile([BCH, CG, NCAPS * OCH], FP32, tag="stg")
                nc.scalar.copy(stg[:], pg[BCH:128, :, :])
                nc.sync.dma_start(usp[:, c0 : c0 + CG, :, :], stg[:])

            for chunk in range(NCHUNK):
                if chunk > 0:
                    u = upool.tile([128, C, OCH], FP32, tag="u")
                    nc.sync.dma_start(u[0:BCH, :, :], usp[:, :, 0, :])
                    nc.sync.dma_start(u[BCH:128, :, :], usp[:, :, 1, :])

                for it in range(ITERATIONS):
                    # --- s = sum_c coeff * u  (coeff == 0.5 on iteration 0) ---
                    for seg in range(NSEG):
                        gp = seg < GP_SEGS
                        eng = nc.gpsimd if gp else nc.vector
                        ek = "g" if gp else "v"
                        cs = seg * CSEG
                        useg = u[:, cs : cs + CSEG, :]
                        if it == 0:
                            src = useg
                            w = scrpool.tile([128, CSEG, OCH], FP32, tag=f"w{ek}")
                        else:
                            w = scrpool.tile([128, CSEG, OCH], FP32, tag=f"w{ek}")
                            cb = (
                                coeff[:, cs : cs + CSEG]
                                .unsqueeze(2)
                                .broadcast_to((128, CSEG, OCH))
                            )
                            eng.tensor_tensor(w[:], useg, cb, op=MULT)
                            src = w[:]
                        ta = scrpool.tile([128, CSEG // 2, OCH], FP32, tag=f"ta{ek}")
                        _tree_halve(eng, src, 1, [ta, w], partial[:, seg : seg + 1, :])
                    # partial (128, 16, O) -> s
                    nc.vector.tensor_tensor(
                        pc8[:], partial[:, 0:8, :], partial[:, 8:16, :], op=ADD
                    )
                    nc.vector.tensor_tensor(pa[:], pc8[:, 0:4, :], pc8[:, 4:8, :], op=ADD)
                    nc.vector.tensor_tensor(pb[:], pa[:, 0:2, :], pa[:, 2:4, :], op=ADD)
                    nc.vector.tensor_tensor(
                        s[:].unsqueeze(1), pb[:, 0:1, :], pb[:, 1:2, :], op=ADD
                    )
                    if it == 0:
                        nc.vector.tensor_scalar(s[:], s[:], 0.5, None, op0=MULT)

                    # --- squash: v = s * sq / ((1+sq) sqrt(sq)) ---
                    nc.vector.tensor_tensor(s2[:], s[:], s[:], op=MULT)
                    nc.vector.tensor_reduce(sq[:], s2[:], axis=mybir.AxisListType.X, op=ADD)
                    nc.vector.tensor_scalar(aa[:], sq[:], 1.0, None, op0=ADD)
                    nc.scalar.activation(rr[:], sq[:], AF.Sqrt)
                    nc.vector.tensor_tensor(den[:], aa[:], rr[:], op=MULT)
                    nc.vector.reciprocal(rec[:], den[:])
                    nc.vector.tensor_tensor(mm[:], sq[:], rec[:], op=MULT)
                    nc.vector.tensor_scalar(v[:], s[:], mm[:], None, op0=MULT)

                    if it == ITERATIONS - 1:
                        break

                    # --- y[(n,b),c] = sum_o u * v ---
                    vb = v[:].unsqueeze(1).broadcast_to((128, CSEG, OCH))
                    for seg in range(NSEG):
                        gp = seg < GP_SEGS
                        eng = nc.gpsimd if gp else nc.vector
                        ek = "g" if gp else "v"
                        cs = seg * CSEG
                        w = scrpool.tile([128, CSEG, OCH], FP32, tag=f"w{ek}")
                        eng.tensor_tensor(w[:], u[:, cs : cs + CSEG, :], vb, op=MULT)
                        ta = scrpool.tile([128, CSEG, OCH // 2], FP32, tag=f"ta{ek}")
                        cur = _tree_halve(eng, w[:], 2, [ta, w], None, stop_at=2)
                        eng.tensor_tensor(
                            y0[:, cs : cs + CSEG].unsqueeze(2),
                            cur[0:BCH, :, 0:1],
                            cur[0:BCH, :, 1:2],
                            op=ADD,
                        )
                        eng.tensor_tensor(
                            y1[:, cs : cs + CSEG].unsqueeze(2),
                            cur[BCH:128, :, 0:1],
                            cur[BCH:128, :, 1:2],
                            op=ADD,
                        )
                    # --- d += y0 - y1 ; coeff = sigmoid(+/- d) ---
                    if it == 0:
                        nc.vector.tensor_tensor(d[:], y0[:], y1[:], op=SUB)
                    else:
                        nc.vector.tensor_tensor(dy[:], y0[:], y1[:], op=SUB)
                        nc.vector.tensor_tensor(d[:], d[:], dy[:], op=ADD)
                    nc.scalar.activation(coeff[0:BCH, :], d[:], AF.Sigmoid)
                    nc.scalar.activation(coeff[BCH:128, :], d[:], AF.Sigmoid, scale=-1.0)

                # --- output: v (p=(n,b64), f=o) -> out[b, n, o] ---
                if debug_dump and chunk == 0:
                    nc.sync.dma_start(dbg_u[:], u[:])
                bs = chunk * BCH
                nc.sync.dma_start(out[bs : bs + BCH, 0, :], v[0:BCH, :])
                nc.sync.dma_start(out[bs : bs + BCH, 1, :], v[BCH:128, :])

    if split_waits:
        _split_sync_waits(nc)
    return nc


def _prep_inputs(x, W):
    x = np.ascontiguousarray(x, dtype=np.float32)
    W0 = np.ascontiguousarray(W.reshape(NCAPS, C, OCH, ICH), dtype=np.float32)
    # wt[h, i, c, n*64+o] = W0[n, c, o, h*128+i]
    w4 = W0.reshape(NCAPS, C, OCH, KH, 128)
    wt = np.ascontiguousarray(w4.transpose(3, 4, 1, 0, 2)).reshape(
        KH, 128, C, NCAPS * OCH
    )
    wth = wt.astype(BF16NP)
    wtl = (wt - wth.astype(np.float32)).astype(BF16NP)
    xth_cores, xtl_cores = [], []
    for k in range(NCORES):
        xc = x[k * BPC : (k + 1) * BPC]  # (128, C, ICH)
        x5 = xc.reshape(BPC, C, KH, 128)
        xt = np.ascontiguousarray(x5.transpose(2, 3, 1, 0))  # (h,i,c,b)
        xh = xt.astype(BF16NP)
        xlo = (xt - xh.astype(np.float32)).astype(BF16NP)
        xth_cores.append(xh)
        xtl_cores.append(xlo)
    return xth_cores, xtl_cores, wth, wtl


_NC_CACHE = {}


def kernel(x, W):
    global LAST_EXEC_NS
    _install_profile_hook()
    if "nc" not in _NC_CACHE:
        _NC_CACHE["nc"] = build_kernel()
    nc = _NC_CACHE["nc"]
    xth_cores, xtl_cores, wth, wtl = _prep_inputs(np.asarray(x), np.asarray(W))
    in_maps = [
        {"xth": xth_cores[k], "xtl": xtl_cores[k], "wth": wth, "wtl": wtl}
        for k in range(NCORES)
    ]
    trace = bool(os.environ.get("CAPS_TRACE"))
    res = run_bass_kernel_spmd(nc, in_maps, list(range(NCORES)), trace=trace)
    LAST_EXEC_NS = res.exec_time_ns
    out = np.concatenate([res.results[k]["out"] for k in range(NCORES)], axis=0)
    return out.astype(np.float32)


# revision 22
# speedup vs baseline: 1.4439x; 1.4042x over previous
"""CapsNet routing layer (nn_CapsLayer) on 8 Trainium2 NeuronCores.

reference:
    u_hat = einsum("ncoi,bci->bnco", W[0], x)         # B,N,C,O = 1024,2,512,64
    3 dynamic-routing iterations (softmax over n, weighted sum over c,
    squash, agreement update); returns v from iteration 3.

Strategy (in-caps sharded, hardcoded shapes):
  - 8 cores x 64 in-caps, every core sees the full batch. Per-core DMA
    is 64 MiB of x + 8 MiB of W (resident in SBUF) instead of the
    128+ MiB a batch shard would need (W replicated).
  - GEMM is a bf16 hi/lo x3 decomposition (xh*Wh + xh*Wl + xl*Wh, fp32
    PSUM accumulate): ~1e-5 relative accuracy at bf16 PE rate with
    fast-weight-load (128 stationary columns = one 128-sample chunk).
  - The batch is processed as 8 pipelined chunks of 128 samples; each
    chunk's u_hat tile (p=b128, f=(n,c,o), 4 MiB fp32) stays in SBUF for
    all 3 routing iterations. Chunk k+1's GEMM overlaps chunk k's
    routing.
  - The weighted sum over in-caps needs a cross-core reduction: one
    64 KB AllReduce per (chunk, iteration) = 24 total, overlapped with
    other chunks' compute. Every core then computes identical v
    (redundantly) and writes the full output; the host takes core 0's.
  - softmax over N=2 collapses to sigmoid(b0-b1); iteration 1 uses the
    exact coefficients 0.5; iteration 3 skips the dead y/b update.
    Routing is single-instruction TT/TR passes on DVE (GPSIMD offload
    measured net-negative: SBUF port contention stretches DVE slices).
"""
import os
import sys
import types

sys.path.insert(0, "/opt/trn_rl_repo")

import numpy as np
import ml_dtypes
import concourse.bass as bass
import concourse.mybir as mybir
import concourse.tile as tile
from concourse.bass_utils import run_bass_kernel_spmd

BF16NP = ml_dtypes.bfloat16

B, NCAPS, C, ICH, OCH = 1024, 2, 512, 256, 64
ITERATIONS = 3
NCORES = 8
CPC = C // NCORES            # in-caps per core = 64
NBCH = 8                     # batch chunks
BCH = B // NBCH              # samples per chunk = 128
KH = 2                       # K halves (ICH = 2*128)
CG = 8                       # c's per GEMM/DMA group

FP32 = mybir.dt.float32
BF16 = mybir.dt.bfloat16
ADD = mybir.AluOpType.add
MULT = mybir.AluOpType.mult
SUB = mybir.AluOpType.subtract
AF = mybir.ActivationFunctionType
AX = mybir.AxisListType

LAST_EXEC_NS = None


def _install_profile_hook():
    """antenv.axon_hooks is absent in this image; recreate it so
    run_bass_kernel_spmd(trace=True)/BASS_TRACE can report exec_time_ns."""
    if "antenv.axon_hooks" in sys.modules:
        return
    mod = types.ModuleType("antenv.axon_hooks")
    mod._hook = None
    mod.set_axon_ntff_profile_hook = lambda h: setattr(mod, "_hook", h)
    mod.get_axon_ntff_profile_hook = lambda: mod._hook
    sys.modules["antenv.axon_hooks"] = mod
    try:
        from trn_agent_boot.trn_boot import _ntff_profile_via_ctypes

        hook = _ntff_profile_via_ctypes("/opt/axon/libaxon_pjrt.so")
        if hook is not None:
            mod._hook = hook
    except Exception:
        pass


def _split_sync_waits(nc, max_waits=1):
    """walrus setupSyncWait rejects instructions with more than one sem
    wait; hoist extras onto same-engine InstNoOp's placed just before."""
    for f in nc.m.functions:
        for bb in f.blocks:
            out = []
            changed = False
            for inst in bb.instructions:
                si = inst.sync_info
                waits = list(si.on_wait) if si is not None and si.on_wait else []
                if len(waits) > max_waits:
                    extra, keep = waits[:-max_waits], waits[-max_waits:]
                    for g, w in enumerate(extra):
                        out.append(
                            mybir.InstNoOp(
                                name=f"{inst.name}_wsplit{g}",
                                engine=inst.engine,
                                bass_nofuse=True,
                                sync_info=mybir.SyncInfo(on_wait=[w], on_update=[]),
                            )
                        )
                    inst.sync_info = mybir.SyncInfo(
                        on_wait=keep,
                        on_update=list(si.on_update) if si.on_update else [],
                    )
                    changed = True
                out.append(inst)
            if changed:
                bb.instructions = out


def build_kernel(split_waits=True):
    nc = bass.Bass(
        "TRN2", target_bir_lowering=False, debug=False, num_devices=NCORES
    )
    # x shard: [h, i, bchunk, c, b] bf16 (hi and lo)
    xth = nc.dram_tensor("xth", [KH, 128, NBCH, CPC, BCH], BF16, kind="ExternalInput").ap()
    xtl = nc.dram_tensor("xtl", [KH, 128, NBCH, CPC, BCH], BF16, kind="ExternalInput").ap()
    # W shard: [h, i, c, (n,o)] bf16 (hi and lo)
    wth = nc.dram_tensor("wth", [KH, 128, CPC, NCAPS * OCH], BF16, kind="ExternalInput").ap()
    wtl = nc.dram_tensor("wtl", [KH, 128, CPC, NCAPS * OCH], BF16, kind="ExternalInput").ap()
    out = nc.dram_tensor("out", [B, NCAPS, OCH], FP32, kind="ExternalOutput").ap()

    with tile.TileContext(nc) as tc:
        with (
            tc.tile_pool(name="xin", bufs=3) as xpool,
            tc.tile_pool(name="psum", bufs=3, space="PSUM") as pspool,
            tc.tile_pool(name="ubuf", bufs=2) as upool,
            tc.tile_pool(name="dram", bufs=4, space="DRAM") as drpool,
        ):
            # resident W: [h][hi/lo] tiles (128i, c*no)
            wsb = {}
            for h in range(KH):
                for hl, src in (("h", wth), ("l", wtl)):
                    t = nc.alloc_sbuf_tensor(f"w{hl}{h}", [128, CPC * NCAPS * OCH], BF16).ap()
                    nc.sync.dma_start(t[:], src[h].rearrange("i c f -> i (c f)"))
                    wsb[(h, hl)] = t

            # static routing tensors
            w = nc.alloc_sbuf_tensor("wscr", [128, NCAPS, CPC, OCH], FP32).ap()
            d_all = nc.alloc_sbuf_tensor("d_all", [128, NBCH, CPC], FP32).ap()
            coeff = nc.alloc_sbuf_tensor("coeff", [128, NCAPS, CPC], FP32).ap()
            sp = nc.alloc_sbuf_tensor("sp", [128, NCAPS, OCH], FP32).ap()
            ss = nc.alloc_sbuf_tensor("ss", [128, NCAPS, OCH], FP32).ap()
            s2 = nc.alloc_sbuf_tensor("s2", [128, NCAPS, OCH], FP32).ap()
            sq = nc.alloc_sbuf_tensor("sq", [128, NCAPS], FP32).ap()
            rr = nc.alloc_sbuf_tensor("rr", [128, NCAPS], FP32).ap()
            den = nc.alloc_sbuf_tensor("den", [128, NCAPS], FP32).ap()
            rec = nc.alloc_sbuf_tensor("rec", [128, NCAPS], FP32).ap()
            mmv = nc.alloc_sbuf_tensor("mmv", [128, NCAPS], FP32).ap()
            v = nc.alloc_sbuf_tensor("v", [128, NCAPS, OCH], FP32).ap()
            y = nc.alloc_sbuf_tensor("y", [128, NCAPS, CPC], FP32).ap()
            dd = nc.alloc_sbuf_tensor("dd", [128, CPC], FP32).ap()

            for bk in range(NBCH):
                u = upool.tile([128, NCAPS, CPC, OCH], FP32, tag="u")

                # ---- GEMM for this chunk ----
                for cg in range(CPC // CG):
                    c0 = cg * CG
                    xt = {}
                    for h in range(KH):
                        for hl, src in (("h", xth), ("l", xtl)):
                            t = xpool.tile([128, CG, BCH], BF16, tag=f"x{hl}{h}")
                            nc.sync.dma_start(t[:], src[h, :, bk, c0 : c0 + CG, :])
                            xt[(h, hl)] = t
                    pg = pspool.tile([BCH, CG, NCAPS * OCH], FP32, tag="pg")
                    for j in range(CG):
                        c = c0 + j
                        terms = []
                        for h in range(KH):
                            wslice_h = wsb[(h, "h")][
                                :, c * NCAPS * OCH : (c + 1) * NCAPS * OCH
                            ]
                            wslice_l = wsb[(h, "l")][
                                :, c * NCAPS * OCH : (c + 1) * NCAPS * OCH
                            ]
                            terms.append((xt[(h, "h")][:, j, :], wslice_h))
                            terms.append((xt[(h, "h")][:, j, :], wslice_l))
                            terms.append((xt[(h, "l")][:, j, :], wslice_h))
                        for ti, (sta, mov) in enumerate(terms):
                            nc.tensor.matmul(
                                pg[:, j, :],
                                lhsT=sta,
                                rhs=mov,
                                start=(ti == 0),
                                stop=(ti == len(terms) - 1),
                            )
                    # PSUM (b, (c,n,o)) -> u (b, (n,c,o)): strided ACT copy
                    nc.scalar.copy(
                        u[:, :, c0 : c0 + CG, :],
                        pg[:].rearrange("b c (n o) -> b n c o", n=NCAPS),
                    )

                # ---- routing for this chunk ----
                d = d_all[:, bk, :]
                for it in range(ITERATIONS):
                    # s_partial = sum_c coeff * u   (coeff = 0.5 exactly on it 0)
                    if it == 0:
                        nc.vector.tensor_reduce(
                            sp[:].unsqueeze(3),
                            u[:].transpose([0, 1, 3, 2]),
                            axis=AX.X,
                            op=ADD,
                        )
                        nc.vector.tensor_scalar(sp[:], sp[:], 0.5, None, op0=MULT)
                    else:
                        cb = coeff[:].unsqueeze(3).broadcast_to((128, NCAPS, CPC, OCH))
                        nc.vector.tensor_tensor(w[:], u[:], cb, op=MULT)
                        nc.vector.tensor_reduce(
                            sp[:].unsqueeze(3),
                            w[:].transpose([0, 1, 3, 2]),
                            axis=AX.X,
                            op=ADD,
                        )
                    # AllReduce s_partial across the 8 c-shards
                    bi = drpool.tile([128, NCAPS * OCH], FP32, tag="bi")
                    bo = drpool.tile([128, NCAPS * OCH], FP32, tag="bo")
                    nc.sync.dma_start(bi[:], sp[:].rearrange("p n o -> p (n o)"))
                    nc.gpsimd.collective_compute(
                        "AllReduce",
                        ADD,
                        replica_groups=[list(range(NCORES))],
                        ins=[bi[:].opt()],
                        outs=[bo[:].opt()],
                    )
                    nc.sync.dma_start(ss[:].rearrange("p n o -> p (n o)"), bo[:])

                    # squash: v = ss * sq / ((1+sq) sqrt(sq))
                    nc.vector.tensor_tensor(s2[:], ss[:], ss[:], op=MULT)
                    nc.vector.tensor_reduce(sq[:].unsqueeze(2), s2[:], axis=AX.X, op=ADD)
                    nc.scalar.activation(rr[:], sq[:], AF.Sqrt)
                    nc.vector.tensor_scalar(den[:], sq[:], 1.0, None, op0=ADD)
                    nc.vector.tensor_tensor(den[:], den[:], rr[:], op=MULT)
                    nc.vector.reciprocal(rec[:], den[:])
                    nc.vector.tensor_tensor(mmv[:], sq[:], rec[:], op=MULT)
                    vb2 = mmv[:].unsqueeze(2).broadcast_to((128, NCAPS, OCH))
                    nc.vector.tensor_tensor(v[:], ss[:], vb2, op=MULT)

                    if it == ITERATIONS - 1:
                        break

                    # y = sum_o u * v
                    vb = v[:].unsqueeze(2).broadcast_to((128, NCAPS, CPC, OCH))
                    nc.vector.tensor_tensor(w[:], u[:], vb, op=MULT)
                    nc.vector.tensor_reduce(y[:].unsqueeze(3), w[:], axis=AX.X, op=ADD)
                    # d += y0 - y1 ; coeff = sigmoid(+/-d)
                    nc.vector.tensor_tensor(dd[:], y[:, 0, :], y[:, 1, :], op=SUB)
                    if it == 0:
                        nc.vector.tensor_copy(d, dd[:])
                    else:
                        nc.vector.tensor_tensor(d, d, dd[:], op=ADD)
                    nc.scalar.activation(coeff[:, 0, :], d, AF.Sigmoid)
                    nc.scalar.activation(coeff[:, 1, :], d, AF.Sigmoid, scale=-1.0)

                # every core writes the (identical) chunk result
                nc.sync.dma_start(out[bk * BCH : (bk + 1) * BCH, :, :], v[:])

    if split_waits:
        _split_sync_waits(nc)
    return nc


def _prep_inputs(x, W):
    x = np.ascontiguousarray(x, dtype=np.float32)
    W0 = np.ascontiguousarray(W.reshape(NCAPS, C, OCH, ICH), dtype=np.float32)
    xth_cores, xtl_cores, wth_cores, wtl_cores = [], [], [], []
    for k in range(NCORES):
        cs = k * CPC
        xc = x[:, cs : cs + CPC, :]  # (B, 64, 256)
        x6 = xc.reshape(NBCH, BCH, CPC, KH, 128)
        xt = np.ascontiguousarray(x6.transpose(3, 4, 0, 2, 1))  # (h,i,bk,c,b)
        xh = xt.astype(BF16NP)
        xlo = (xt - xh.astype(np.float32)).astype(BF16NP)
        xth_cores.append(xh)
        xtl_cores.append(xlo)
        Wc = W0[:, cs : cs + CPC]  # (2, 64, 64, 256)
        w5 = Wc.reshape(NCAPS, CPC, OCH, KH, 128)
        wt = np.ascontiguousarray(w5.transpose(3, 4, 1, 0, 2)).reshape(
            KH, 128, CPC, NCAPS * OCH
        )
        wh = wt.astype(BF16NP)
        wlo = (wt - wh.astype(np.float32)).astype(BF16NP)
        wth_cores.append(wh)
        wtl_cores.append(wlo)
    return xth_cores, xtl_cores, wth_cores, wtl_cores


_NC_CACHE = {}


def kernel(x, W):
    global LAST_EXEC_NS
    _install_profile_hook()
    if "nc" not in _NC_CACHE:
        _NC_CACHE["nc"] = build_kernel()
    nc = _NC_CACHE["nc"]
    xth, xtl, wth, wtl = _prep_inputs(np.asarray(x), np.asarray(W))
    in_maps = [
        {"xth": xth[k], "xtl": xtl[k], "wth": wth[k], "wtl": wtl[k]}
        for k in range(NCORES)
    ]
    trace = bool(os.environ.get("CAPS_TRACE"))
    res = run_bass_kernel_spmd(nc, in_maps, list(range(NCORES)), trace=trace)
    LAST_EXEC_NS = res.exec_time_ns
    return res.results[0]["out"].astype(np.float32)
